# revision 10
# baseline (speedup 1.0000x reference)
"""Alignment generator (length regulator) on 8 TRN2 NeuronCores.

out[b, f, j] = 1.0  iff  starts[b,j] <= f < ends[b,j]  (ends = cumsum(dur))

Each output row out[b, f, :] is one-hot at token_id[b, f] =
searchsorted(ends[b], f, side='right') (or all-zero when no token covers
frame f). The host computes token_id from the tiny [32, 512] duration input;
each core then generates its 4-row slab of the ~256MB output with one DVE
tensor_scalar(is_equal) per [128, 512] tile and streams it out in HWDGE
DMAs. The kernel is DMA/HBM write bound.

Trace-derived SDMA model this kernel is built around:
  * One DMA's partition dim is split EVENLY across engines:
    engines_used = largest divisor of n_partitions <= 16, engine k taking
    the k-th contiguous partition block (128p -> 16 engines x 8;
    120p -> 15 engines x 8, engine 15 IDLE; 112p -> 16 x 7; 8p -> E0-7).
    92 partitions would fan out to only 4 engines - 3x slower end to end.
  * A per-partition contiguous run is cut into equal power-of-2 pieces
    <= 16KB. 16KB descriptors run at ~25 GB/s/engine; 8KB only ~20.
    So per-instruction per-partition bytes should be a multiple of 16KB.
  * Engine 15 is ~17% slower than engines 0-14 (hence with a uniform
    128-partition layout it straggles ~12us past the stream).

Layout (per output row, three DRAM regions, each affine in partition id):
  R1 frames [0, 120*S15):        partition p<120 covers [p*S15, +S15)
  R2 frames [R1, R1+112*S16):    partition p<112 covers [R1+p*S16, +S16)
  R3 frames [R2e, R2e+8*LGT):    partition p>=120 covers [R2e+(p-120)*LGT)
R2 moves ~89% of bytes via 112-partition instrs (16 engines, E15 at 7/8
the per-engine share of a 128p instr = its measured speed deficit); R1
(120p, E15-free) and R3 (8p -> E0-7) absorb the rest so every engine
finishes together just under the ~87us HBM-window floor.

Raw Bass (no Tile): this walrus build only allows a single sync-wait per
compute/DMA instruction, so all synchronization is explicit standalone
wait_ge with a ring of NBUF buffers and one completion semaphore per buffer
slot; per-slot DMA-count bookkeeping makes "slot's previous DMAs fully
drained" provable from a 16*count threshold (every DMA increments its sem
by 16 regardless of partition count, cf. concourse/zero.py). Main-region
instrs alternate between the SP and ACT HWDGE rings to hide per-ring
instruction-boundary gaps.

Sharding: pure data parallelism, batch dim 32 -> 4 rows per core; no
collectives.
"""

import math
from contextlib import ExitStack

import numpy as np

import concourse.bass as bass
import concourse.mybir as mybir
from concourse.bass_utils import run_bass_kernel_spmd

N_CORES = 8
B = 32          # batch
T = 512         # tokens
P = 128         # SBUF partitions
N1 = 120        # R1 partitions (15 engines, E15-free)
N2 = 112        # R2 partitions (16 engines x 7)
NL = 8          # R3 light partitions 120..127 (engines 0-7)
GROUP = 8       # span steps per chunk: 8*T*4B = 16KB single descriptors
NBUF = 4        # output buffer ring slots

_nc_cache: dict[tuple[int, int], bass.Bass] = {}


def _geometry(m_frames: int):
    """Pick (S15, S16, LGT) region step counts. Cost = slice-model max
    engine time (ns/step weights from traced 16KB/8KB descriptor rates),
    tie-break on padded size."""
    best = None
    for s16 in range(8, math.ceil(m_frames / N2) + 8, 8):
        for s15 in range(0, 9):
            lgt = max(0, math.ceil((m_frames - N2 * s16 - N1 * s15) / NL))
            if lgt > s16:
                continue
            m_pad = N1 * s15 + N2 * s16 + NL * lgt
            if m_pad < m_frames:
                continue
            t15 = 7 * s16 * 95.5
            t814 = 7 * s16 * 81.6 + 8 * s15 * 103.0
            t07 = t814 + lgt * 104.0
            thbm = m_pad * 5.30  # 2048B/step / 386GB/s -> ns per row
            cand = (max(t15, t814, t07, thbm), m_pad, s15, s16, lgt)
            if best is None or cand < best:
                best = cand
    _, m_pad, s15, s16, lgt = best
    assert m_pad >= m_frames
    return s15, s16, lgt, m_pad


def _row_chunks(s15: int, s16: int):
    """Column chunks of one row: [S15] (R1+R3 round), then S16 split into
    GROUP-step pieces (16KB descriptors)."""
    sizes = [s15] if s15 else []
    rem = s16
    while rem > 0:
        g = min(GROUP, rem)
        sizes.append(g)
        rem -= g
    return sizes


def _ramp_chunks(ncols: int):
    """Row-0 chunk sizes: small ramp pieces first so the first output DMA
    is issued as soon as possible after the input lands -- the DMA stream
    is the bottleneck and every ns it starts earlier is a ns off the
    kernel."""
    sizes, acc = [], 0
    for x in [1, 1, 2, 4, 4] + [GROUP] * 1000:
        if acc >= ncols:
            break
        g = min(x, ncols - acc)
        sizes.append(g)
        acc += g
    return sizes


def _rounds(s15: int, s16: int, b_loc: int):
    """(row, first_col, n_cols) for every chunk of every row."""
    sizes = _row_chunks(s15, s16)
    rounds = []
    for b in range(b_loc):
        g0 = 0
        for g in _ramp_chunks(s15 + s16) if b == 0 else sizes:
            rounds.append((b, g0, g))
            g0 += g
    return rounds


def _round_segs(s15, s16, lgt, g0, g):
    """Segments (region, region_step0, n_steps) this round's cols cover."""
    segs = []
    a, b = g0, g0 + g
    if a < s15:
        segs.append(("R1", a, min(b, s15) - a))
    if b > s15 and a < s15 + s16:
        a2 = max(a, s15)
        segs.append(("R2", a2 - s15, min(b, s15 + s16) - a2))
    if a < lgt:
        segs.append(("R3", a, min(b, lgt) - a))
    return segs


def _build(m_frames: int, b_loc: int) -> bass.Bass:
    """Per-core Bass graph writing a [b_loc, m_pad, T] padded output slab."""
    s15, s16, lgt, m_pad = _geometry(m_frames)
    ncols = s15 + s16
    rounds = _rounds(s15, s16, b_loc)
    n_rounds = len(rounds)
    segs = [_round_segs(s15, s16, lgt, g0, g) for (_, g0, g) in rounds]
    ndma = [len(x) for x in segs]

    r1 = (0, N1 * s15)                    # DRAM row ranges per output row
    r2 = (r1[1], r1[1] + N2 * s16)
    r3 = (r2[1], r2[1] + NL * lgt)
    assert r3[1] == m_pad

    nc = bass.Bass()
    # input column (b*ncols + k) on partition p = token id of the frame
    # that (p, col k) covers; the iota row J (J[p,j] = j) is generated
    # on-device by GpSimd in parallel with this DMA
    tid = nc.declare_dram_parameter(
        "tid", [P, b_loc * ncols], mybir.dt.float32, isOutput=False
    )
    out = nc.declare_dram_parameter(
        "out", [b_loc, m_pad, T], mybir.dt.float32, isOutput=True
    )

    with ExitStack() as ctx:
        sb = ctx.enter_context(
            nc.sbuf_tensor("sb", [P, b_loc * ncols], mybir.dt.float32)
        )
        Jsb = ctx.enter_context(nc.sbuf_tensor("J", [P, T], mybir.dt.float32))
        bufs = [
            ctx.enter_context(
                nc.sbuf_tensor(f"buf{s}", [P, GROUP * T], mybir.dt.float32)
            )
            for s in range(NBUF)
        ]
        in_sem = ctx.enter_context(nc.semaphore("in_sem"))
        j_sem = ctx.enter_context(nc.semaphore("j_sem"))
        c_sem = ctx.enter_context(nc.semaphore("c_sem"))
        d_sems = [ctx.enter_context(nc.semaphore(f"d_sem{s}")) for s in range(NBUF)]
        block = ctx.enter_context(nc.Block())

        @block.gpsimd
        def _(gpsimd):
            # values 0..511 are exact in fp32
            gpsimd.iota(
                Jsb[:, :],
                pattern=[[1, T]],
                base=0,
                channel_multiplier=0,
                allow_small_or_imprecise_dtypes=True,
            ).then_inc(j_sem, 1)

        def issue(eng, r):
            """All DMA segments of round r on engine eng's HWDGE ring."""
            b, g0, g = rounds[r]
            s = r % NBUF
            eng.wait_ge(c_sem, r + 1)
            for reg, st0, n in segs[r]:
                if reg == "R1":
                    lo, hi, np_ = r1[0], r1[1], N1
                elif reg == "R2":
                    lo, hi, np_ = r2[0], r2[1], N2
                else:
                    lo, hi, np_ = r3[0], r3[1], NL
                # buf col offset of this segment's first step within the round
                bc0 = (st0 + s15 - g0) if reg == "R2" else (st0 - g0)
                dview = out[b][lo:hi].rearrange("(p i) t -> p (i t)", p=np_)[
                    :, st0 * T : (st0 + n) * T
                ]
                sbv = (bufs[s][:N1] if reg == "R1"
                       else bufs[s][:N2] if reg == "R2"
                       else bufs[s][N1:])
                eng.dma_start(
                    out=dview, in_=sbv[:, bc0 * T : (bc0 + n) * T]
                ).then_inc(d_sems[s], 16)

        @block.sync
        def _(sync):
            sync.dma_start(out=sb[:, :], in_=tid[:, :]).then_inc(in_sem, 16)
            for r in range(0, n_rounds, 2):
                issue(sync, r)
            # all output bytes landed before the NEFF may finish
            tot = [0] * NBUF
            for r in range(n_rounds):
                tot[r % NBUF] += ndma[r]
            for s in range(NBUF):
                if tot[s]:
                    sync.wait_ge(d_sems[s], 16 * tot[s])

        @block.scalar
        def _(scalar):
            for r in range(1, n_rounds, 2):
                issue(scalar, r)

        @block.vector
        def _(vector):
            vector.wait_ge(j_sem, 1)
            vector.wait_ge(in_sem, 16)
            cum = [0] * NBUF  # DMAs issued into slot s before this round
            for r, (b, g0, g) in enumerate(rounds):
                s = r % NBUF
                if r >= NBUF:
                    # slot's previous DMAs (round r-NBUF) fully drained
                    vector.wait_ge(d_sems[s], 16 * cum[s])
                last = None
                for k in range(g):
                    col = b * ncols + g0 + k
                    last = nc.vector.tensor_scalar(
                        out=bufs[s][:, k * T : (k + 1) * T],
                        in0=Jsb[:, :],
                        scalar1=sb[:, col : col + 1],
                        scalar2=None,
                        op0=mybir.AluOpType.is_equal,
                    )
                last.then_inc(c_sem, 1)
                cum[s] += ndma[r]

    return nc


def _token_ids(dur: np.ndarray, m_pad: int) -> np.ndarray:
    """tid[b, f] = index of the token whose frame interval contains f,
    or T (out of range -> all-zero output row) when no token covers f."""
    ends = np.cumsum(dur.astype(np.int64), axis=1)
    frames = np.arange(m_pad, dtype=np.int64)
    tid = np.empty((dur.shape[0], m_pad), dtype=np.float32)
    for b in range(dur.shape[0]):
        tid[b] = np.searchsorted(ends[b], frames, side="right")
    return tid


def _col_frames(s15: int, s16: int, lgt: int, m_pad: int):
    """frame index [P, ncols] each (partition, col) covers, and a mask of
    (partition, col) cells that are outside the partition's spans."""
    ncols = s15 + s16
    ps = np.arange(P)[:, None]
    k = np.arange(ncols)[None, :]
    frame = np.zeros((P, ncols), dtype=np.int64)
    mask = np.zeros((P, ncols), dtype=bool)
    in_r1 = k < s15
    in_r2 = ~in_r1
    # R1: partitions 0..119, cols [0, s15)
    frame = np.where(in_r1, ps * s15 + k, frame)
    mask |= in_r1 & (ps >= N1)
    # R2: partitions 0..111, cols [s15, s15+s16)
    frame = np.where(in_r2, N1 * s15 + ps * s16 + (k - s15), frame)
    mask |= in_r2 & (ps >= N2)
    # R3: partitions 120..127, cols [0, lgt) override R1 mapping
    in_r3 = (ps >= N1) & (k < lgt)
    frame = np.where(
        in_r3, N1 * s15 + N2 * s16 + (ps - N1) * lgt + k, frame
    )
    mask &= ~in_r3
    return np.minimum(frame, m_pad - 1), mask


def _prepare(duration_predictor_output: np.ndarray, max_frames):
    """Host-side prep: token ids, per-core input maps, cached Bass graph."""
    dur = np.asarray(duration_predictor_output)
    m_frames = int(max_frames)
    b_loc = B // N_CORES
    s15, s16, lgt, m_pad = _geometry(m_frames)

    tid = _token_ids(dur, m_pad)  # [B, m_pad] float32

    key = (m_frames, b_loc)
    nc = _nc_cache.get(key)
    if nc is None:
        nc = _build(m_frames, b_loc)
        _nc_cache[key] = nc

    idx, mask = _col_frames(s15, s16, lgt, m_pad)

    in_maps = []
    for i in range(N_CORES):
        cols = []
        for b in range(b_loc):
            tb = tid[i * b_loc + b][idx]          # [P, ncols]
            tb[mask] = float(T)
            cols.append(tb)
        in_maps.append({"tid": np.ascontiguousarray(np.concatenate(cols, axis=1))})
    return nc, in_maps


def kernel(duration_predictor_output: np.ndarray, max_frames) -> np.ndarray:
    dur = np.asarray(duration_predictor_output)
    m_frames = int(max_frames)
    if m_frames <= 0:
        return np.zeros((dur.shape[0], 0, dur.shape[1]), dtype=np.float32)

    nc, in_maps = _prepare(dur, m_frames)
    res = run_bass_kernel_spmd(nc, in_maps, core_ids=list(range(N_CORES)))
    full = np.concatenate([res.results[i]["out"] for i in range(N_CORES)], axis=0)
    return np.ascontiguousarray(full[:, :m_frames, :])


# revision 13
# speedup vs baseline: 1.0419x; 1.0419x over previous
"""Alignment generator (length regulator) on 8 TRN2 NeuronCores.

out[b, f, j] = 1.0  iff  starts[b,j] <= f < ends[b,j]  (ends = cumsum(dur))

Each output row out[b, f, :] is one-hot at token_id[b, f] =
searchsorted(ends[b], f, side='right') (or all-zero when no token covers
frame f). The host computes token_id from the tiny [32, 512] duration input;
each core then generates its 4-row slab of the ~256MB output with one DVE
tensor_scalar(is_equal) per [128, 512] tile and streams it out in HWDGE
DMAs. The kernel is DMA/HBM write bound.

Trace-derived SDMA model this kernel is built around:
  * One DMA's partition dim is split EVENLY across engines:
    engines_used = largest divisor of n_partitions <= 16, engine k taking
    the k-th contiguous partition block (128p -> 16 engines x 8;
    120p -> 15 engines x 8, engine 15 IDLE; 112p -> 16 x 7; 8p -> E0-7).
    92 partitions would fan out to only 4 engines - 3x slower end to end.
  * A per-partition contiguous run is cut into equal power-of-2 pieces
    <= 16KB. 16KB descriptors run at ~25 GB/s/engine; 8KB only ~20.
    So per-instruction per-partition bytes should be a multiple of 16KB.
  * Engine 15 is ~17% slower than engines 0-14 (hence with a uniform
    128-partition layout it straggles ~12us past the stream).

Layout (per output row, three DRAM regions, each affine in partition id):
  R1 frames [0, 120*S15):        partition p<120 covers [p*S15, +S15)
  R2 frames [R1, R1+112*S16):    partition p<112 covers [R1+p*S16, +S16)
  R3 frames [R2e, R2e+8*LGT):    partition p>=120 covers [R2e+(p-120)*LGT)
R2 moves ~89% of bytes via 112-partition instrs (16 engines, E15 at 7/8
the per-engine share of a 128p instr = its measured speed deficit); R1
(120p, E15-free) and R3 (8p -> E0-7) absorb the rest so every engine
finishes together just under the ~87us HBM-window floor.

Raw Bass (no Tile): this walrus build only allows a single sync-wait per
compute/DMA instruction, so all synchronization is explicit standalone
wait_ge with a ring of NBUF buffers and one completion semaphore per buffer
slot; per-slot DMA-count bookkeeping makes "slot's previous DMAs fully
drained" provable from a 16*count threshold (every DMA increments its sem
by 16 regardless of partition count, cf. concourse/zero.py). Main-region
instrs alternate between the SP and ACT HWDGE rings to hide per-ring
instruction-boundary gaps.

Sharding: pure data parallelism, batch dim 32 -> 4 rows per core; no
collectives.
"""

import math
from contextlib import ExitStack

import numpy as np

import concourse.bass as bass
import concourse.mybir as mybir
from concourse.bass_utils import run_bass_kernel_spmd

N_CORES = 8
B = 32          # batch
T = 512         # tokens
P = 128         # SBUF partitions
N1 = 120        # R1 partitions (15 engines, E15-free)
N2 = 112        # R2 partitions (16 engines x 7)
NL = 8          # R3 light partitions 120..127 (engines 0-7)
GROUP = 8       # span steps per chunk: 8*T*4B = 16KB single descriptors
NBUF = 4        # output buffer ring slots

_nc_cache: dict[tuple[int, int], bass.Bass] = {}


def _geometry(m_frames: int):
    """Pick (S15, S16, LGT) region step counts. Cost = slice-model max
    engine time (ns/step weights from traced 16KB/8KB descriptor rates),
    tie-break on padded size."""
    best = None
    for s16 in range(8, math.ceil(m_frames / N2) + 8, 8):
        for s15 in range(0, 9):
            lgt = max(0, math.ceil((m_frames - N2 * s16 - N1 * s15) / NL))
            if lgt > s16:
                continue
            m_pad = N1 * s15 + N2 * s16 + NL * lgt
            if m_pad < m_frames:
                continue
            t15 = 7 * s16 * 95.5
            t814 = 7 * s16 * 81.6 + 8 * s15 * 103.0
            t07 = t814 + lgt * 104.0
            thbm = m_pad * 5.30  # 2048B/step / 386GB/s -> ns per row
            cand = (max(t15, t814, t07, thbm), m_pad, s15, s16, lgt)
            if best is None or cand < best:
                best = cand
    _, m_pad, s15, s16, lgt = best
    assert m_pad >= m_frames
    return s15, s16, lgt, m_pad


def _row_chunks(s15: int, s16: int):
    """Column chunks of one row: [S15] (R1+R3 round), then S16 split into
    GROUP-step pieces (16KB descriptors)."""
    sizes = [s15] if s15 else []
    rem = s16
    while rem > 0:
        g = min(GROUP, rem)
        sizes.append(g)
        rem -= g
    return sizes


def _ramp_chunks(ncols: int):
    """Row-0 chunk sizes: small ramp pieces first so the first output DMA
    is issued as soon as possible after the input lands -- the DMA stream
    is the bottleneck and every ns it starts earlier is a ns off the
    kernel."""
    sizes, acc = [], 0
    for x in [1, 1, 2, 4, 4] + [GROUP] * 1000:
        if acc >= ncols:
            break
        g = min(x, ncols - acc)
        sizes.append(g)
        acc += g
    return sizes


def _rounds(s15: int, s16: int, b_loc: int):
    """(row, first_col, n_cols) for every chunk of every row."""
    sizes = _row_chunks(s15, s16)
    rounds = []
    for b in range(b_loc):
        g0 = 0
        for g in _ramp_chunks(s15 + s16) if b == 0 else sizes:
            rounds.append((b, g0, g))
            g0 += g
    return rounds


def _round_segs(s15, s16, lgt, g0, g):
    """Segments (region, region_step0, n_steps) this round's cols cover."""
    segs = []
    a, b = g0, g0 + g
    if a < s15:
        segs.append(("R1", a, min(b, s15) - a))
    if b > s15 and a < s15 + s16:
        a2 = max(a, s15)
        segs.append(("R2", a2 - s15, min(b, s15 + s16) - a2))
    if a < lgt:
        segs.append(("R3", a, min(b, lgt) - a))
    return segs


def _build(m_frames: int, b_loc: int) -> bass.Bass:
    """Per-core Bass graph writing a [b_loc, m_pad, T] padded output slab."""
    s15, s16, lgt, m_pad = _geometry(m_frames)
    ncols = s15 + s16
    rounds = _rounds(s15, s16, b_loc)
    n_rounds = len(rounds)
    segs = [_round_segs(s15, s16, lgt, g0, g) for (_, g0, g) in rounds]
    ndma = [len(x) for x in segs]

    r1 = (0, N1 * s15)                    # DRAM row ranges per output row
    r2 = (r1[1], r1[1] + N2 * s16)
    r3 = (r2[1], r2[1] + NL * lgt)
    assert r3[1] == m_pad

    nc = bass.Bass()
    # input column (b*ncols + k) on partition p = token id of the frame
    # that (p, col k) covers; the iota row J (J[p,j] = j) is generated
    # on-device by GpSimd in parallel with this DMA
    tid = nc.declare_dram_parameter(
        "tid", [P, b_loc * ncols], mybir.dt.float32, isOutput=False
    )
    out = nc.declare_dram_parameter(
        "out", [b_loc, m_pad, T], mybir.dt.float32, isOutput=True
    )

    with ExitStack() as ctx:
        sb = ctx.enter_context(
            nc.sbuf_tensor("sb", [P, b_loc * ncols], mybir.dt.float32)
        )
        Jsb = ctx.enter_context(nc.sbuf_tensor("J", [P, T], mybir.dt.float32))
        bufs = [
            ctx.enter_context(
                nc.sbuf_tensor(f"buf{s}", [P, GROUP * T], mybir.dt.float32)
            )
            for s in range(NBUF)
        ]
        in_sem = ctx.enter_context(nc.semaphore("in_sem"))
        j_sem = ctx.enter_context(nc.semaphore("j_sem"))
        c_sem = ctx.enter_context(nc.semaphore("c_sem"))
        d_sems = [ctx.enter_context(nc.semaphore(f"d_sem{s}")) for s in range(NBUF)]
        block = ctx.enter_context(nc.Block())

        @block.gpsimd
        def _(gpsimd):
            # values 0..511 are exact in fp32
            gpsimd.iota(
                Jsb[:, :],
                pattern=[[1, T]],
                base=0,
                channel_multiplier=0,
                allow_small_or_imprecise_dtypes=True,
            ).then_inc(j_sem, 1)

        def issue(eng, r, regs):
            """Round r's DMA segments in `regs` on engine eng's HWDGE ring."""
            b, g0, g = rounds[r]
            s = r % NBUF
            mine = [x for x in segs[r] if x[0] in regs]
            if not mine:
                return
            eng.wait_ge(c_sem, r + 1)
            for reg, st0, n in mine:
                if reg == "R1":
                    lo, hi, np_ = r1[0], r1[1], N1
                elif reg == "R2":
                    lo, hi, np_ = r2[0], r2[1], N2
                else:
                    lo, hi, np_ = r3[0], r3[1], NL
                # buf col offset of this segment's first step within the round
                bc0 = (st0 + s15 - g0) if reg == "R2" else (st0 - g0)
                dview = out[b][lo:hi].rearrange("(p i) t -> p (i t)", p=np_)[
                    :, st0 * T : (st0 + n) * T
                ]
                sbv = (bufs[s][:N1] if reg == "R1"
                       else bufs[s][:N2] if reg == "R2"
                       else bufs[s][N1:])
                eng.dma_start(
                    out=dview, in_=sbv[:, bc0 * T : (bc0 + n) * T]
                ).then_inc(d_sems[s], 16)

        @block.sync
        def _(sync):
            sync.dma_start(out=sb[:, :], in_=tid[:, :]).then_inc(in_sem, 16)
            for r in range(n_rounds):
                issue(sync, r, ("R1", "R2"))
            # all output bytes landed before the NEFF may finish
            tot = [0] * NBUF
            for r in range(n_rounds):
                tot[r % NBUF] += ndma[r]
            for s in range(NBUF):
                if tot[s]:
                    sync.wait_ge(d_sems[s], 16 * tot[s])

        @block.scalar
        def _(scalar):
            for r in range(n_rounds):
                issue(scalar, r, ("R3",))

        @block.vector
        def _(vector):
            vector.wait_ge(j_sem, 1)
            vector.wait_ge(in_sem, 16)
            cum = [0] * NBUF  # DMAs issued into slot s before this round
            for r, (b, g0, g) in enumerate(rounds):
                s = r % NBUF
                if r >= NBUF:
                    # slot's previous DMAs (round r-NBUF) fully drained
                    vector.wait_ge(d_sems[s], 16 * cum[s])
                last = None
                for k in range(g):
                    col = b * ncols + g0 + k
                    last = nc.vector.tensor_scalar(
                        out=bufs[s][:, k * T : (k + 1) * T],
                        in0=Jsb[:, :],
                        scalar1=sb[:, col : col + 1],
                        scalar2=None,
                        op0=mybir.AluOpType.is_equal,
                    )
                last.then_inc(c_sem, 1)
                cum[s] += ndma[r]

    return nc


def _token_ids(dur: np.ndarray, m_pad: int) -> np.ndarray:
    """tid[b, f] = index of the token whose frame interval contains f,
    or T (out of range -> all-zero output row) when no token covers f."""
    ends = np.cumsum(dur.astype(np.int64), axis=1)
    frames = np.arange(m_pad, dtype=np.int64)
    tid = np.empty((dur.shape[0], m_pad), dtype=np.float32)
    for b in range(dur.shape[0]):
        tid[b] = np.searchsorted(ends[b], frames, side="right")
    return tid


def _col_frames(s15: int, s16: int, lgt: int, m_pad: int):
    """frame index [P, ncols] each (partition, col) covers, and a mask of
    (partition, col) cells that are outside the partition's spans."""
    ncols = s15 + s16
    ps = np.arange(P)[:, None]
    k = np.arange(ncols)[None, :]
    frame = np.zeros((P, ncols), dtype=np.int64)
    mask = np.zeros((P, ncols), dtype=bool)
    in_r1 = k < s15
    in_r2 = ~in_r1
    # R1: partitions 0..119, cols [0, s15)
    frame = np.where(in_r1, ps * s15 + k, frame)
    mask |= in_r1 & (ps >= N1)
    # R2: partitions 0..111, cols [s15, s15+s16)
    frame = np.where(in_r2, N1 * s15 + ps * s16 + (k - s15), frame)
    mask |= in_r2 & (ps >= N2)
    # R3: partitions 120..127, cols [0, lgt) override R1 mapping
    in_r3 = (ps >= N1) & (k < lgt)
    frame = np.where(
        in_r3, N1 * s15 + N2 * s16 + (ps - N1) * lgt + k, frame
    )
    mask &= ~in_r3
    return np.minimum(frame, m_pad - 1), mask


def _prepare(duration_predictor_output: np.ndarray, max_frames):
    """Host-side prep: token ids, per-core input maps, cached Bass graph."""
    dur = np.asarray(duration_predictor_output)
    m_frames = int(max_frames)
    b_loc = B // N_CORES
    s15, s16, lgt, m_pad = _geometry(m_frames)

    tid = _token_ids(dur, m_pad)  # [B, m_pad] float32

    key = (m_frames, b_loc)
    nc = _nc_cache.get(key)
    if nc is None:
        nc = _build(m_frames, b_loc)
        _nc_cache[key] = nc

    idx, mask = _col_frames(s15, s16, lgt, m_pad)

    in_maps = []
    for i in range(N_CORES):
        cols = []
        for b in range(b_loc):
            tb = tid[i * b_loc + b][idx]          # [P, ncols]
            tb[mask] = float(T)
            cols.append(tb)
        in_maps.append({"tid": np.ascontiguousarray(np.concatenate(cols, axis=1))})
    return nc, in_maps


def kernel(duration_predictor_output: np.ndarray, max_frames) -> np.ndarray:
    dur = np.asarray(duration_predictor_output)
    m_frames = int(max_frames)
    if m_frames <= 0:
        return np.zeros((dur.shape[0], 0, dur.shape[1]), dtype=np.float32)

    nc, in_maps = _prepare(dur, m_frames)
    res = run_bass_kernel_spmd(nc, in_maps, core_ids=list(range(N_CORES)))
    full = np.concatenate([res.results[i]["out"] for i in range(N_CORES)], axis=0)
    return np.ascontiguousarray(full[:, :m_frames, :])


# revision 14
# speedup vs baseline: 1.1737x; 1.1265x over previous
"""Alignment generator (length regulator) on 8 TRN2 NeuronCores.

out[b, f, j] = 1.0  iff  starts[b,j] <= f < ends[b,j]  (ends = cumsum(dur))

Each output row out[b, f, :] is one-hot at token_id[b, f] =
searchsorted(ends[b], f, side='right') (or all-zero when no token covers
frame f). The host computes token_id from the tiny [32, 512] duration input;
each core then generates its 4-row slab of the ~256MB output with one DVE
tensor_scalar(is_equal) per [128, 512] tile and streams it out in HWDGE
DMAs. The kernel is DMA/HBM write bound.

Trace-derived SDMA model this kernel is built around:
  * One DMA's partition dim is split EVENLY across engines:
    engines_used = largest divisor of n_partitions <= 16, engine k taking
    the k-th contiguous partition block (128p -> 16 engines x 8;
    120p -> 15 engines x 8, engine 15 IDLE; 92p -> only 4 engines, 3x
    slower end to end).
  * Partitions-per-engine must be a multiple of 8: SBUF ports serve
    4-partition groups, so 112p instrs (16 x 7) make adjacent engines
    share ports and all descriptors stretch ~25% (measured).
  * A per-partition contiguous run is cut into equal power-of-2 pieces
    <= 16KB; 16KB descriptors run at ~25 GB/s/engine, 8KB only ~20.
    So chunks are 8 span-steps = 16KB.
  * Engine 15 is ~17% slower than engines 0-14 (with a uniform
    128-partition layout it straggles ~12us past the end of the stream).

Layout (per output row, two DRAM regions, each affine in partition id):
  RS frames [0, 128*SS):          all partitions p: [p*SS, +SS)
  RN frames [128*SS, +120*SN):    p<120: [128*SS + p*SN, +SN)
RS moves via 128-partition instrs (all 16 engines), RN via 120-partition
instrs (engine 15 idle). SS tunes engine 15's share below its speed
deficit so every engine drains just under the ~87us HBM-window floor.

Raw Bass (no Tile): this walrus build only allows a single sync-wait per
compute/DMA instruction, so all synchronization is explicit standalone
wait_ge with a ring of NBUF buffers and one completion semaphore per buffer
slot (per-slot sems make "slot's previous DMA fully drained" provable from
a 16*m threshold; every DMA increments its sem by 16 regardless of
partition count, cf. concourse/zero.py).

Sharding: pure data parallelism, batch dim 32 -> 4 rows per core; no
collectives.
"""

import math
from contextlib import ExitStack

import numpy as np

import concourse.bass as bass
import concourse.mybir as mybir
from concourse.bass_utils import run_bass_kernel_spmd

N_CORES = 8
B = 32          # batch
T = 512         # tokens
P = 128         # SBUF partitions
NN = 120        # RN partitions (15 engines, engine-15-free)
GROUP = 8       # span steps per chunk: 8*T*4B = 16KB single descriptors
NBUF = 4        # output buffer ring slots

_nc_cache: dict[tuple[int, int], bass.Bass] = {}

# measured per-step (2KB) engine costs, ns: 16KB descs on E0-14 / E15
_C14, _C15 = 81.6, 95.5
# leftover-chunk descriptor cost by step count (single small descriptor)
_CREM = {0: 0, 1: 108, 2: 211, 3: 310, 4: 412, 5: 470, 6: 530, 7: 590}


def _geometry(m_frames: int):
    """Pick (SS, SN) region step counts minimizing the slice-model max
    engine time (and HBM window), tie-break on padded size."""

    def eng_cost(steps, c):
        full, rem = divmod(steps, GROUP)
        return full * GROUP * c + _CREM[rem] * (c / _C14)

    best = None
    hi = math.ceil(m_frames / NN) + 1
    for ss in range(0, hi + 1):
        for sn in range(0, hi + 1):
            m_pad = P * ss + NN * sn
            if m_pad < m_frames or (best and m_pad > best[1] + 256):
                continue
            t15 = 8 * eng_cost(ss, _C15)
            t14 = 8 * (eng_cost(ss, _C14) + eng_cost(sn, _C14))
            thbm = m_pad * 5.30  # 2048B/step / 386GB/s -> ns per row
            cand = (max(t15, t14, thbm), m_pad, ss, sn)
            if best is None or cand < best:
                best = cand
    _, m_pad, ss, sn = best
    assert m_pad >= m_frames
    return ss, sn, m_pad


def _chunks(steps: int):
    sizes = []
    while steps > 0:
        g = min(GROUP, steps)
        sizes.append(g)
        steps -= g
    return sizes


def _rounds(ss: int, sn: int, b_loc: int):
    """(row, first_col, n_cols); cols [0,ss) are RS steps, [ss,ss+sn) RN.
    No chunk crosses the region boundary. Row 0's first chunk is split
    1,1,2,4 so the first output DMA is issued as soon as possible after
    the input lands -- the DMA stream is the bottleneck and every ns it
    starts earlier is a ns off the kernel."""
    sizes = _chunks(ss) + _chunks(sn)
    rounds = []
    for b in range(b_loc):
        row = sizes
        if b == 0 and sizes:
            ramp, acc = [], 0
            for x in [1, 1, 2, 4, 8]:
                if acc >= sizes[0]:
                    break
                g = min(x, sizes[0] - acc)
                ramp.append(g)
                acc += g
            row = ramp + sizes[1:]
        g0 = 0
        for g in row:
            rounds.append((b, g0, g))
            g0 += g
    return rounds


def _build(m_frames: int, b_loc: int) -> bass.Bass:
    """Per-core Bass graph writing a [b_loc, m_pad, T] padded output slab."""
    ss, sn, m_pad = _geometry(m_frames)
    ncols = ss + sn
    rounds = _rounds(ss, sn, b_loc)
    n_rounds = len(rounds)

    rs_end = P * ss          # DRAM row ranges per output row
    assert rs_end + NN * sn == m_pad

    nc = bass.Bass()
    # input column (b*ncols + k) on partition p = token id of the frame
    # that (p, col k) covers; the iota row J (J[p,j] = j) is generated
    # on-device by GpSimd in parallel with this DMA
    tid = nc.declare_dram_parameter(
        "tid", [P, b_loc * ncols], mybir.dt.float32, isOutput=False
    )
    out = nc.declare_dram_parameter(
        "out", [b_loc, m_pad, T], mybir.dt.float32, isOutput=True
    )

    with ExitStack() as ctx:
        sb = ctx.enter_context(
            nc.sbuf_tensor("sb", [P, b_loc * ncols], mybir.dt.float32)
        )
        Jsb = ctx.enter_context(nc.sbuf_tensor("J", [P, T], mybir.dt.float32))
        bufs = [
            ctx.enter_context(
                nc.sbuf_tensor(f"buf{s}", [P, GROUP * T], mybir.dt.float32)
            )
            for s in range(NBUF)
        ]
        in_sem = ctx.enter_context(nc.semaphore("in_sem"))
        j_sem = ctx.enter_context(nc.semaphore("j_sem"))
        c_sem = ctx.enter_context(nc.semaphore("c_sem"))
        d_sems = [ctx.enter_context(nc.semaphore(f"d_sem{s}")) for s in range(NBUF)]
        block = ctx.enter_context(nc.Block())

        @block.gpsimd
        def _(gpsimd):
            # values 0..511 are exact in fp32
            gpsimd.iota(
                Jsb[:, :],
                pattern=[[1, T]],
                base=0,
                channel_multiplier=0,
                allow_small_or_imprecise_dtypes=True,
            ).then_inc(j_sem, 1)

        def issue(eng, r):
            b, g0, g = rounds[r]
            s = r % NBUF
            eng.wait_ge(c_sem, r + 1)
            if g0 < ss:  # RS chunk: all 128 partitions, 16 engines
                dview = out[b][:rs_end].rearrange("(p i) t -> p (i t)", p=P)[
                    :, g0 * T : (g0 + g) * T
                ]
                sbv = bufs[s][:, : g * T]
            else:        # RN chunk: partitions 0:120, engine 15 idle
                st0 = g0 - ss
                dview = out[b][rs_end:].rearrange("(p i) t -> p (i t)", p=NN)[
                    :, st0 * T : (st0 + g) * T
                ]
                sbv = bufs[s][:NN, : g * T]
            eng.dma_start(out=dview, in_=sbv).then_inc(d_sems[s], 16)

        @block.sync
        def _(sync):
            sync.dma_start(out=sb[:, :], in_=tid[:, :]).then_inc(in_sem, 16)
            for r in range(n_rounds):
                issue(sync, r)
            # all output bytes landed before the NEFF may finish
            for s in range(NBUF):
                uses = len(range(s, n_rounds, NBUF))
                if uses:
                    sync.wait_ge(d_sems[s], 16 * uses)

        @block.vector
        def _(vector):
            vector.wait_ge(j_sem, 1)
            vector.wait_ge(in_sem, 16)
            for r, (b, g0, g) in enumerate(rounds):
                s = r % NBUF
                if r >= NBUF:
                    # slot's previous DMA (round r-NBUF) fully drained
                    vector.wait_ge(d_sems[s], 16 * (r // NBUF))
                last = None
                for k in range(g):
                    col = b * ncols + g0 + k
                    last = nc.vector.tensor_scalar(
                        out=bufs[s][:, k * T : (k + 1) * T],
                        in0=Jsb[:, :],
                        scalar1=sb[:, col : col + 1],
                        scalar2=None,
                        op0=mybir.AluOpType.is_equal,
                    )
                last.then_inc(c_sem, 1)

    return nc


def _token_ids(dur: np.ndarray, m_pad: int) -> np.ndarray:
    """tid[b, f] = index of the token whose frame interval contains f,
    or T (out of range -> all-zero output row) when no token covers f."""
    ends = np.cumsum(dur.astype(np.int64), axis=1)
    frames = np.arange(m_pad, dtype=np.int64)
    tid = np.empty((dur.shape[0], m_pad), dtype=np.float32)
    for b in range(dur.shape[0]):
        tid[b] = np.searchsorted(ends[b], frames, side="right")
    return tid


def _col_frames(ss: int, sn: int, m_pad: int):
    """frame index [P, ncols] each (partition, col) covers + mask of cells
    outside the partition's spans (light partitions beyond RS)."""
    ncols = ss + sn
    ps = np.arange(P)[:, None]
    k = np.arange(ncols)[None, :]
    in_rs = k < ss
    frame = np.where(in_rs, ps * ss + k, P * ss + ps * sn + (k - ss))
    mask = (~in_rs) & (ps >= NN)
    return np.minimum(frame, max(m_pad - 1, 0)), mask


def _prepare(duration_predictor_output: np.ndarray, max_frames):
    """Host-side prep: token ids, per-core input maps, cached Bass graph."""
    dur = np.asarray(duration_predictor_output)
    m_frames = int(max_frames)
    b_loc = B // N_CORES
    ss, sn, m_pad = _geometry(m_frames)

    tid = _token_ids(dur, m_pad)  # [B, m_pad] float32

    key = (m_frames, b_loc)
    nc = _nc_cache.get(key)
    if nc is None:
        nc = _build(m_frames, b_loc)
        _nc_cache[key] = nc

    idx, mask = _col_frames(ss, sn, m_pad)

    in_maps = []
    for i in range(N_CORES):
        cols = []
        for b in range(b_loc):
            tb = tid[i * b_loc + b][idx]          # [P, ncols]
            tb[mask] = float(T)
            cols.append(tb)
        in_maps.append({"tid": np.ascontiguousarray(np.concatenate(cols, axis=1))})
    return nc, in_maps


def kernel(duration_predictor_output: np.ndarray, max_frames) -> np.ndarray:
    dur = np.asarray(duration_predictor_output)
    m_frames = int(max_frames)
    if m_frames <= 0:
        return np.zeros((dur.shape[0], 0, dur.shape[1]), dtype=np.float32)

    nc, in_maps = _prepare(dur, m_frames)
    res = run_bass_kernel_spmd(nc, in_maps, core_ids=list(range(N_CORES)))
    full = np.concatenate([res.results[i]["out"] for i in range(N_CORES)], axis=0)
    return np.ascontiguousarray(full[:, :m_frames, :])
